# revision 18
# baseline (speedup 1.0000x reference)
"""ChaRNN (LSTM w/ teacher forcing) Trainium2 kernel.

B=4096, T=32, D=512 (per-step feature), R=512 (rnn size), C=256 (classes).
Data-parallel over batch: 8 cores x 512 rows each.

Device-side layout is fully "transposed" (batch on the free dim):
  hT, cT: [R, Bc] as SBUF [128, 4, 512]
  zT = Wx1^T @ featT + Wh^T @ hT (+ prev-char row of Wx2)
so no on-device transposes are needed anywhere.  The host pre-transposes
f_pool into featT [T, D, Bc] and post-transposes the outputs (layout prep
only; all FLOPs happen on device).  Matmuls run in float32r (full-rate
fp32 PE mode).

Teacher forcing means the prev-char one-hot contribution is a known row
gather of Wx2: done with dma_gather(transpose=True) from HBM in bf16
(index 256 = appended zeros row, used at t=0), then added to zT on DVE.
"""

import numpy as np

B, T, D, R, C = 4096, 32, 512, 512, 256
NCORES = 8
BC = B // NCORES  # 512 batch rows per core
NM = (4 * R) // 128  # 16 gate row-tiles
KF = D // 128  # 4 K-chunks for the feature matmul
KO = C // 128  # 2 K-chunks for the one-hot matmul (non-gather variant)
KH = R // 128  # 4 K-chunks for the recurrent matmul
NCI = C // 128  # 2 logits row-tiles
IDXW = 16  # dma_gather index wrap width

_CACHED = {}


def _build_program(t_steps=T, use_gather=True):
    import concourse.bacc as bacc
    import concourse.bass as bass
    import concourse.mybir as mybir
    import concourse.tile as tile

    f32 = mybir.dt.float32
    f32r = mybir.dt.float32r
    bf16 = mybir.dt.bfloat16
    i16 = mybir.dt.int16
    AF = mybir.ActivationFunctionType

    nc = bacc.Bacc()

    # ---- I/O ----
    featT_d = nc.dram_tensor("featT", [t_steps, D, BC], f32r, kind="ExternalInput")
    wx1_d = nc.dram_tensor("wx1", [D, 4 * R], f32r, kind="ExternalInput")
    wh_d = nc.dram_tensor("wh", [R, 4 * R], f32r, kind="ExternalInput")
    smw_d = nc.dram_tensor("smw", [R, C], f32r, kind="ExternalInput")
    bcol_d = nc.dram_tensor("bcol", [128, NM], f32, kind="ExternalInput")
    smbcol_d = nc.dram_tensor("smbcol", [128, NCI], f32, kind="ExternalInput")
    if use_gather:
        wx2g_d = nc.dram_tensor("wx2g", [C + 16, 4 * R], bf16, kind="ExternalInput")
        pidx_d = nc.dram_tensor(
            "pidx", [128, t_steps * (BC // IDXW)], i16, kind="ExternalInput"
        )
    else:
        wx2_d = nc.dram_tensor("wx2", [C, 4 * R], f32r, kind="ExternalInput")
        prevT_d = nc.dram_tensor("prevT", [t_steps, BC], f32r, kind="ExternalInput")
        iotac_d = nc.dram_tensor("iotac", [128, NCI], f32, kind="ExternalInput")
        ones_d = nc.dram_tensor("ones", [1, 128], f32r, kind="ExternalInput")

    outl_d = nc.dram_tensor("outl", [t_steps, C, BC], f32, kind="ExternalOutput")
    hT_d = nc.dram_tensor("hT_out", [R, BC], f32r, kind="ExternalOutput")
    cT_d = nc.dram_tensor("cT_out", [R, BC], f32, kind="ExternalOutput")

    NIDX = BC // IDXW  # 32 idx columns per step

    with tile.TileContext(nc) as tc:
        with (
            tc.tile_pool(name="weights", bufs=1) as wpool,
            tc.tile_pool(name="state", bufs=2) as spool,
            tc.tile_pool(name="feat", bufs=2) as fpool,
            tc.tile_pool(name="oh", bufs=2) as ohpool,
            tc.tile_pool(name="gates", bufs=2) as gpool,
            tc.tile_pool(name="outs", bufs=2) as opool,
            tc.tile_pool(name="zpsum", bufs=6, space="PSUM") as zpool,
            tc.tile_pool(name="lpsum", bufs=2, space="PSUM") as lpool,
        ):
            # ---- load weights (once); split per K-chunk so the first
            # matmuls only wait for their own chunk ----
            wx1 = wpool.tile([128, KF, 4 * R], f32r)
            for k in range(KF):
                eng = nc.sync if k % 2 == 0 else nc.scalar
                eng.dma_start(wx1[:, k, :], wx1_d[k * 128 : (k + 1) * 128, :])
            def load_ft(t, split=False):
                ft = fpool.tile([128, KF, BC], f32r, tag="ft")
                if split:
                    for k in range(KF):
                        eng = nc.sync if k % 2 == 0 else nc.scalar
                        eng.dma_start(
                            ft[:, k, :], featT_d[t, k * 128 : (k + 1) * 128, :]
                        )
                else:
                    nc.sync.dma_start(
                        ft[:], featT_d[t].rearrange("(k p) b -> p k b", p=128)
                    )
                return ft

            ft0 = load_ft(0)  # prefetch step-0 features before wh
            wh = wpool.tile([128, KH, 4 * R], f32r)
            for k in range(KH):
                eng = nc.sync if k % 2 == 1 else nc.scalar
                eng.dma_start(wh[:, k, :], wh_d[k * 128 : (k + 1) * 128, :])
            smw = wpool.tile([128, KH, C], f32r)
            nc.scalar.dma_start(smw[:], smw_d.rearrange("(k p) m -> p k m", p=128))
            bcol = wpool.tile([128, NM], f32)
            nc.scalar.dma_start(bcol[:], bcol_d[:])
            smbcol = wpool.tile([128, NCI], f32)
            nc.scalar.dma_start(smbcol[:], smbcol_d[:])
            if use_gather:
                pidx = wpool.tile([128, t_steps * NIDX], i16)
                nc.scalar.dma_start(pidx[:], pidx_d[:])
            else:
                wx2 = wpool.tile([128, KO, 4 * R], f32r)
                for k in range(KO):
                    nc.sync.dma_start(wx2[:, k, :], wx2_d[k * 128 : (k + 1) * 128, :])
                iotac = wpool.tile([128, NCI], f32)
                nc.sync.dma_start(iotac[:], iotac_d[:])
                ones = wpool.tile([1, 128], f32r)
                nc.sync.dma_start(ones[:], ones_d[:])

            # ---- initial state (h0 = 0 * wx1-chunk, producing f32r) ----
            hT = spool.tile([128, KH, BC], f32r, tag="hT")
            cT = spool.tile([128, KH, BC], f32, tag="cT")
            nc.vector.tensor_scalar(
                hT[:].rearrange("p a b -> p (a b)"),
                wx1[:, 0, :],
                0.0,
                None,
                mybir.AluOpType.mult,
            )
            nc.vector.memset(cT[:], 0.0)

            for t in range(t_steps):
                if use_gather:
                    # prev-char rows of Wx2, transposed by the gather DMA:
                    # z2t[p, m, j] = Wx2g[idx_j, m*128+p]  (bf16)
                    z2t = ohpool.tile([128, NM, BC], bf16, tag="z2t")
                    nc.gpsimd.dma_gather(
                        out_ap=z2t[:],
                        in_ap=wx2g_d[:],
                        idxs_ap=pidx[:, t * NIDX : (t + 1) * NIDX],
                        num_idxs=BC,
                        num_idxs_reg=BC,
                        elem_size=4 * R,
                        transpose=True,
                    )
                else:
                    prow = ohpool.tile([1, BC], f32r, tag="prow")
                    nc.sync.dma_start(prow[:], prevT_d[t : t + 1, :])
                    pb = lpool.tile([128, BC], f32, tag="pb")
                    nc.tensor.matmul(pb[:], ones[:], prow[:], start=True, stop=True)
                    oh = ohpool.tile([128, KO, BC], f32r, tag="oh")
                    for ci in range(KO):
                        nc.vector.tensor_scalar(
                            oh[:, ci, :],
                            pb[:],
                            iotac[:, ci : ci + 1],
                            None,
                            mybir.AluOpType.is_equal,
                        )

                # per-step image features, pre-transposed on host
                ft = ft0 if t == 0 else load_ft(t)

                # ---- zT accumulation: 16 row-tiles of [128, Bc] ----
                zts = []
                for m in range(NM):
                    zt = zpool.tile([128, BC], f32, tag="zt")
                    zts.append(zt)
                    for k in range(KF):
                        nc.tensor.matmul(
                            zt[:],
                            wx1[:, k, m * 128 : (m + 1) * 128],
                            ft[:, k, :],
                            start=(k == 0),
                            stop=False,
                        )
                    if not use_gather:
                        for k in range(KO):
                            nc.tensor.matmul(
                                zt[:],
                                wx2[:, k, m * 128 : (m + 1) * 128],
                                oh[:, k, :],
                                start=False,
                                stop=False,
                            )
                # recurrent part (needs previous h)
                for m in range(NM):
                    for k in range(KH):
                        nc.tensor.matmul(
                            zts[m][:],
                            wh[:, k, m * 128 : (m + 1) * 128],
                            hT[:, k, :],
                            start=False,
                            stop=(k == KH - 1),
                        )

                # ---- gates (Keras order i, f, g, o along rows) ----
                hT_new = spool.tile([128, KH, BC], f32r, tag="hT")
                cT_new = spool.tile([128, KH, BC], f32, tag="cT")
                for j in range(KH):  # 4 chunks of 128 R-rows
                    mi, mf, mg, mo = j, 4 + j, 8 + j, 12 + j
                    it = gpool.tile([128, BC], f32, tag="it")
                    ftg = gpool.tile([128, BC], f32, tag="ftg")
                    gt = gpool.tile([128, BC], f32, tag="gt")
                    ot = gpool.tile([128, BC], f32, tag="ot")
                    if use_gather:
                        # z = zT(psum) + gathered prev-char row, then gate fn
                        zi = gpool.tile([128, BC], f32, tag="zi")
                        zf = gpool.tile([128, BC], f32, tag="zf")
                        zg = gpool.tile([128, BC], f32, tag="zg")
                        zo = gpool.tile([128, BC], f32, tag="zo")
                        nc.vector.tensor_add(zi[:], zts[mi][:], z2t[:, mi, :])
                        nc.vector.tensor_add(zf[:], zts[mf][:], z2t[:, mf, :])
                        nc.vector.tensor_add(zg[:], zts[mg][:], z2t[:, mg, :])
                        nc.vector.tensor_add(zo[:], zts[mo][:], z2t[:, mo, :])
                        si, sf, sg, so = zi, zf, zg, zo
                    else:
                        si, sf, sg, so = zts[mi], zts[mf], zts[mg], zts[mo]
                    nc.scalar.activation(
                        it[:], si[:], AF.Sigmoid, bias=bcol[:, mi : mi + 1]
                    )
                    nc.scalar.activation(
                        ftg[:], sf[:], AF.Sigmoid, bias=bcol[:, mf : mf + 1]
                    )
                    nc.scalar.activation(
                        gt[:], sg[:], AF.Tanh, bias=bcol[:, mg : mg + 1]
                    )
                    nc.scalar.activation(
                        ot[:], so[:], AF.Sigmoid, bias=bcol[:, mo : mo + 1]
                    )
                    ig = gpool.tile([128, BC], f32, tag="ig")
                    nc.vector.tensor_mul(ig[:], it[:], gt[:])
                    nc.vector.tensor_mul(cT_new[:, j, :], cT[:, j, :], ftg[:])
                    nc.vector.tensor_add(cT_new[:, j, :], cT_new[:, j, :], ig[:])
                    tct = gpool.tile([128, BC], f32, tag="tct")
                    nc.scalar.activation(tct[:], cT_new[:, j, :], AF.Tanh)
                    nc.vector.tensor_mul(hT_new[:, j, :], ot[:], tct[:])
                hT, cT = hT_new, cT_new

                # ---- logits: [C, Bc] in 2 row-tiles ----
                ol = opool.tile([128, NCI, BC], f32, tag="ol")
                for ci in range(NCI):
                    lg = lpool.tile([128, BC], f32, tag="lg")
                    for k in range(KH):
                        nc.tensor.matmul(
                            lg[:],
                            smw[:, k, ci * 128 : (ci + 1) * 128],
                            hT[:, k, :],
                            start=(k == 0),
                            stop=(k == KH - 1),
                        )
                    nc.scalar.activation(
                        ol[:, ci, :], lg[:], AF.Identity, bias=smbcol[:, ci : ci + 1]
                    )
                nc.sync.dma_start(
                    outl_d[t].rearrange("(c p) b -> p c b", p=128), ol[:]
                )

            # ---- final state out ----
            nc.sync.dma_start(hT_d.rearrange("(k p) b -> p k b", p=128), hT[:])
            nc.sync.dma_start(cT_d.rearrange("(k p) b -> p k b", p=128), cT[:])

    nc.finalize()
    return nc


def _prep_inputs(
    f_pool, ground_truth, Wx, Wh, b, softmax_w, softmax_b, t_steps=T, use_gather=True
):
    """Host-side layout prep + sharding. Returns per-core input maps."""
    f_pool = np.ascontiguousarray(np.asarray(f_pool, dtype=np.float32))
    gt = np.asarray(ground_truth)
    Wx = np.asarray(Wx, dtype=np.float32)
    Wh = np.ascontiguousarray(np.asarray(Wh, dtype=np.float32))
    b = np.asarray(b, dtype=np.float32)
    smw = np.ascontiguousarray(np.asarray(softmax_w, dtype=np.float32))
    smb = np.asarray(softmax_b, dtype=np.float32)

    wx1 = np.ascontiguousarray(Wx[:D])
    bcol = np.ascontiguousarray(b.reshape(NM, 128).T)
    smbcol = np.ascontiguousarray(smb.reshape(NCI, 128).T)

    common = {
        "wx1": wx1,
        "wh": Wh,
        "smw": smw,
        "bcol": bcol,
        "smbcol": smbcol,
    }
    if use_gather:
        import ml_dtypes

        wx2g = np.zeros((C + 16, 4 * R), dtype=ml_dtypes.bfloat16)
        wx2g[:C] = Wx[D : D + C].astype(ml_dtypes.bfloat16)
        common["wx2g"] = wx2g
    else:
        common["wx2"] = np.ascontiguousarray(Wx[D : D + C])
        iotac = (
            np.arange(128, dtype=np.float32)[:, None]
            + 128.0 * np.arange(NCI, dtype=np.float32)[None, :]
        )
        common["iotac"] = np.ascontiguousarray(iotac)
        common["ones"] = np.ones((1, 128), dtype=np.float32)

    NIDX = BC // IDXW
    in_maps = []
    for core in range(NCORES):
        b0 = core * BC
        fp = f_pool[b0 : b0 + BC, :t_steps]  # [Bc, t, D]
        featT = np.ascontiguousarray(fp.transpose(1, 2, 0))  # [t, D, Bc]
        m = dict(common)
        m["featT"] = featT
        if use_gather:
            # idx per step: t=0 -> C (zeros row); else gt[:, t-1]
            idx = np.full((t_steps, BC), C, dtype=np.int16)
            idx[1:] = gt[b0 : b0 + BC, : t_steps - 1].T.astype(np.int16)
            # wrap: pidx[p, t*NIDX + s] = idx[t, s*16 + p] for p<16,
            # replicated to all 8 GPSIMD core groups (16 partitions each)
            w = idx.reshape(t_steps, NIDX, IDXW)  # [t, s, p]
            blk = w.transpose(2, 0, 1).reshape(IDXW, t_steps * NIDX)
            m["pidx"] = np.ascontiguousarray(np.tile(blk, (128 // IDXW, 1)))
        else:
            prevT = np.full((t_steps, BC), -1.0, dtype=np.float32)
            prevT[1:] = gt[b0 : b0 + BC, : t_steps - 1].T.astype(np.float32)
            m["prevT"] = np.ascontiguousarray(prevT)
        in_maps.append(m)
    return in_maps


def _postprocess(results, t_steps=T):
    """Gather per-core outputs back to full [B,T,C], [B,R], [B,R]."""
    seq = np.empty((B, t_steps, C), dtype=np.float32)
    h = np.empty((B, R), dtype=np.float32)
    c = np.empty((B, R), dtype=np.float32)
    for core, res in enumerate(results):
        b0 = core * BC
        seq[b0 : b0 + BC] = res["outl"].transpose(2, 0, 1)
        h[b0 : b0 + BC] = res["hT_out"].T
        c[b0 : b0 + BC] = res["cT_out"].T
    return seq, h, c


def run(inputs, t_steps=T, trace=False, use_gather=True):
    from concourse import bass_utils

    key = (t_steps, use_gather)
    if key not in _CACHED:
        _CACHED[key] = _build_program(t_steps, use_gather)
    nc = _CACHED[key]
    in_maps = _prep_inputs(**inputs, t_steps=t_steps, use_gather=use_gather)
    res = bass_utils.run_bass_kernel_spmd(
        nc, in_maps, core_ids=list(range(NCORES)), trace=trace
    )
    return res


def kernel(**inputs):
    res = run(inputs)
    return _postprocess(res.results)


# revision 19
# speedup vs baseline: 1.6637x; 1.6637x over previous
"""ChaRNN (LSTM w/ teacher forcing) Trainium2 kernel.

B=4096, T=32, D=512 (per-step feature), R=512 (rnn size), C=256 (classes).
Data-parallel over batch: 8 cores x 512 rows each.

Device-side layout is fully "transposed" (batch on the free dim):
  hT, cT: [R, Bc] as SBUF [128, 4, 512]
  zT = Wx1^T @ featT + Wh^T @ hT (+ prev-char row of Wx2)
so no on-device transposes are needed anywhere.  The host pre-transposes
f_pool into featT [T, D, Bc] and post-transposes the outputs (layout prep
only; all FLOPs happen on device).  Matmuls run in float32r (full-rate
fp32 PE mode).

Teacher forcing means the prev-char one-hot contribution is a known row
gather of Wx2: done with dma_gather(transpose=True) from HBM in bf16
(index 256 = appended zeros row, used at t=0), then added to zT on DVE.
"""

import numpy as np

B, T, D, R, C = 4096, 32, 512, 512, 256
NCORES = 8
BC = B // NCORES  # 512 batch rows per core
NM = (4 * R) // 128  # 16 gate row-tiles
KF = D // 128  # 4 K-chunks for the feature matmul
KO = C // 128  # 2 K-chunks for the one-hot matmul (non-gather variant)
KH = R // 128  # 4 K-chunks for the recurrent matmul
NCI = C // 128  # 2 logits row-tiles
IDXW = 16  # dma_gather index wrap width

_CACHED = {}


def _build_program(t_steps=T, use_gather=True):
    import concourse.bacc as bacc
    import concourse.bass as bass
    import concourse.mybir as mybir
    import concourse.tile as tile

    f32 = mybir.dt.float32
    f32r = mybir.dt.float32r
    bf16 = mybir.dt.bfloat16
    i16 = mybir.dt.int16
    AF = mybir.ActivationFunctionType

    nc = bacc.Bacc()

    # ---- I/O ----
    featT_d = nc.dram_tensor("featT", [t_steps, D, BC], f32r, kind="ExternalInput")
    wx1_d = nc.dram_tensor("wx1", [D, 4 * R], f32r, kind="ExternalInput")
    wh_d = nc.dram_tensor("wh", [R, 4 * R], f32r, kind="ExternalInput")
    smw_d = nc.dram_tensor("smw", [R, C], f32r, kind="ExternalInput")
    bcol_d = nc.dram_tensor("bcol", [128, NM], f32, kind="ExternalInput")
    smbcol_d = nc.dram_tensor("smbcol", [128, NCI], f32, kind="ExternalInput")
    if use_gather:
        wx2g_d = nc.dram_tensor("wx2g", [C + 16, 4 * R], bf16, kind="ExternalInput")
        pidx_d = nc.dram_tensor(
            "pidx", [128, t_steps * (BC // IDXW)], i16, kind="ExternalInput"
        )
    else:
        wx2_d = nc.dram_tensor("wx2", [C, 4 * R], f32r, kind="ExternalInput")
        prevT_d = nc.dram_tensor("prevT", [t_steps, BC], f32r, kind="ExternalInput")
        iotac_d = nc.dram_tensor("iotac", [128, NCI], f32, kind="ExternalInput")
        ones_d = nc.dram_tensor("ones", [1, 128], f32r, kind="ExternalInput")

    outl_d = nc.dram_tensor("outl", [t_steps, C, BC], f32, kind="ExternalOutput")
    hT_d = nc.dram_tensor("hT_out", [R, BC], f32r, kind="ExternalOutput")
    cT_d = nc.dram_tensor("cT_out", [R, BC], f32, kind="ExternalOutput")

    NIDX = BC // IDXW  # 32 idx columns per step

    with tile.TileContext(nc) as tc:
        with (
            tc.tile_pool(name="weights", bufs=1) as wpool,
            tc.tile_pool(name="state", bufs=2) as spool,
            tc.tile_pool(name="feat", bufs=2) as fpool,
            tc.tile_pool(name="oh", bufs=2) as ohpool,
            tc.tile_pool(name="gates", bufs=2) as gpool,
            tc.tile_pool(name="outs", bufs=2) as opool,
            tc.tile_pool(name="zpsum", bufs=6, space="PSUM") as zpool,
            tc.tile_pool(name="lpsum", bufs=2, space="PSUM") as lpool,
        ):
            # ---- load weights (once); split per K-chunk so the first
            # matmuls only wait for their own chunk ----
            wx1 = wpool.tile([128, KF, 4 * R], f32r)
            for k in range(KF):
                eng = nc.sync if k % 2 == 0 else nc.scalar
                eng.dma_start(wx1[:, k, :], wx1_d[k * 128 : (k + 1) * 128, :])
            def load_ft(t, split=False):
                ft = fpool.tile([128, KF, BC], f32r, tag="ft")
                if split:
                    for k in range(KF):
                        eng = nc.sync if k % 2 == 0 else nc.scalar
                        eng.dma_start(
                            ft[:, k, :], featT_d[t, k * 128 : (k + 1) * 128, :]
                        )
                else:
                    nc.sync.dma_start(
                        ft[:], featT_d[t].rearrange("(k p) b -> p k b", p=128)
                    )
                return ft

            ft0 = load_ft(0)  # prefetch step-0 features before wh
            wh = wpool.tile([128, KH, 4 * R], f32r)
            for k in range(KH):
                eng = nc.sync if k % 2 == 1 else nc.scalar
                eng.dma_start(wh[:, k, :], wh_d[k * 128 : (k + 1) * 128, :])
            smw = wpool.tile([128, KH, C], f32r)
            nc.scalar.dma_start(smw[:], smw_d.rearrange("(k p) m -> p k m", p=128))
            bcol = wpool.tile([128, NM], f32)
            nc.scalar.dma_start(bcol[:], bcol_d[:])
            smbcol = wpool.tile([128, NCI], f32)
            nc.scalar.dma_start(smbcol[:], smbcol_d[:])
            if use_gather:
                pidx = wpool.tile([128, t_steps * NIDX], i16)
                nc.scalar.dma_start(pidx[:], pidx_d[:])
            else:
                wx2 = wpool.tile([128, KO, 4 * R], f32r)
                for k in range(KO):
                    nc.sync.dma_start(wx2[:, k, :], wx2_d[k * 128 : (k + 1) * 128, :])
                iotac = wpool.tile([128, NCI], f32)
                nc.sync.dma_start(iotac[:], iotac_d[:])
                ones = wpool.tile([1, 128], f32r)
                nc.sync.dma_start(ones[:], ones_d[:])

            # ---- initial state: h0 = c0 = 0, handled by skipping the
            # recurrent matmuls and the zero-row gather at t=0 ----
            hT = None
            cT = spool.tile([128, KH, BC], f32, tag="cT")
            nc.vector.memset(cT[:], 0.0)

            for t in range(t_steps):
                if use_gather and t > 0:
                    # prev-char rows of Wx2, transposed by the gather DMA:
                    # z2t[p, m, j] = Wx2g[idx_j, m*128+p]  (bf16)
                    z2t = ohpool.tile([128, NM, BC], bf16, tag="z2t")
                    nc.gpsimd.dma_gather(
                        out_ap=z2t[:],
                        in_ap=wx2g_d[:],
                        idxs_ap=pidx[:, t * NIDX : (t + 1) * NIDX],
                        num_idxs=BC,
                        num_idxs_reg=BC,
                        elem_size=4 * R,
                        transpose=True,
                    )
                elif use_gather:
                    z2t = None
                else:
                    prow = ohpool.tile([1, BC], f32r, tag="prow")
                    nc.sync.dma_start(prow[:], prevT_d[t : t + 1, :])
                    pb = lpool.tile([128, BC], f32, tag="pb")
                    nc.tensor.matmul(pb[:], ones[:], prow[:], start=True, stop=True)
                    oh = ohpool.tile([128, KO, BC], f32r, tag="oh")
                    for ci in range(KO):
                        nc.vector.tensor_scalar(
                            oh[:, ci, :],
                            pb[:],
                            iotac[:, ci : ci + 1],
                            None,
                            mybir.AluOpType.is_equal,
                        )

                # per-step image features, pre-transposed on host
                ft = ft0 if t == 0 else load_ft(t)

                # ---- zT accumulation: 16 row-tiles of [128, Bc] ----
                zts = []
                for m in range(NM):
                    zt = zpool.tile([128, BC], f32, tag="zt")
                    zts.append(zt)
                    for k in range(KF):
                        nc.tensor.matmul(
                            zt[:],
                            wx1[:, k, m * 128 : (m + 1) * 128],
                            ft[:, k, :],
                            start=(k == 0),
                            stop=(t == 0 and use_gather and k == KF - 1),
                        )
                    if not use_gather:
                        for k in range(KO):
                            nc.tensor.matmul(
                                zt[:],
                                wx2[:, k, m * 128 : (m + 1) * 128],
                                oh[:, k, :],
                                start=False,
                                stop=False,
                            )
                # recurrent part (needs previous h); h(-1) = 0 -> skip at t=0
                if t > 0:
                    for m in range(NM):
                        for k in range(KH):
                            nc.tensor.matmul(
                                zts[m][:],
                                wh[:, k, m * 128 : (m + 1) * 128],
                                hT[:, k, :],
                                start=False,
                                stop=(k == KH - 1),
                            )

                # ---- gates (Keras order i, f, g, o along rows) ----
                hT_new = spool.tile([128, KH, BC], f32r, tag="hT")
                cT_new = spool.tile([128, KH, BC], f32, tag="cT")
                for j in range(KH):  # 4 chunks of 128 R-rows
                    mi, mf, mg, mo = j, 4 + j, 8 + j, 12 + j
                    it = gpool.tile([128, BC], f32, tag="it")
                    ftg = gpool.tile([128, BC], f32, tag="ftg")
                    gt = gpool.tile([128, BC], f32, tag="gt")
                    ot = gpool.tile([128, BC], f32, tag="ot")
                    if use_gather and t > 0:
                        # z = zT(psum) + gathered prev-char row, then gate fn
                        zi = gpool.tile([128, BC], f32, tag="zi")
                        zf = gpool.tile([128, BC], f32, tag="zf")
                        zg = gpool.tile([128, BC], f32, tag="zg")
                        zo = gpool.tile([128, BC], f32, tag="zo")
                        nc.vector.tensor_add(zi[:], zts[mi][:], z2t[:, mi, :])
                        nc.vector.tensor_add(zf[:], zts[mf][:], z2t[:, mf, :])
                        nc.vector.tensor_add(zg[:], zts[mg][:], z2t[:, mg, :])
                        nc.vector.tensor_add(zo[:], zts[mo][:], z2t[:, mo, :])
                        si, sf, sg, so = zi, zf, zg, zo
                    else:
                        si, sf, sg, so = zts[mi], zts[mf], zts[mg], zts[mo]
                    nc.scalar.activation(
                        it[:], si[:], AF.Sigmoid, bias=bcol[:, mi : mi + 1]
                    )
                    nc.scalar.activation(
                        ftg[:], sf[:], AF.Sigmoid, bias=bcol[:, mf : mf + 1]
                    )
                    nc.scalar.activation(
                        gt[:], sg[:], AF.Tanh, bias=bcol[:, mg : mg + 1]
                    )
                    nc.scalar.activation(
                        ot[:], so[:], AF.Sigmoid, bias=bcol[:, mo : mo + 1]
                    )
                    if t == 0:
                        nc.vector.tensor_mul(cT_new[:, j, :], it[:], gt[:])
                    else:
                        ig = gpool.tile([128, BC], f32, tag="ig")
                        nc.vector.tensor_mul(ig[:], it[:], gt[:])
                        nc.vector.tensor_mul(cT_new[:, j, :], cT[:, j, :], ftg[:])
                        nc.vector.tensor_add(cT_new[:, j, :], cT_new[:, j, :], ig[:])
                    tct = gpool.tile([128, BC], f32, tag="tct")
                    nc.scalar.activation(tct[:], cT_new[:, j, :], AF.Tanh)
                    nc.vector.tensor_mul(hT_new[:, j, :], ot[:], tct[:])
                hT, cT = hT_new, cT_new

                # ---- logits: [C, Bc] in 2 row-tiles ----
                ol = opool.tile([128, NCI, BC], f32, tag="ol")
                for ci in range(NCI):
                    lg = lpool.tile([128, BC], f32, tag="lg")
                    for k in range(KH):
                        nc.tensor.matmul(
                            lg[:],
                            smw[:, k, ci * 128 : (ci + 1) * 128],
                            hT[:, k, :],
                            start=(k == 0),
                            stop=(k == KH - 1),
                        )
                    nc.scalar.activation(
                        ol[:, ci, :], lg[:], AF.Identity, bias=smbcol[:, ci : ci + 1]
                    )
                nc.sync.dma_start(
                    outl_d[t].rearrange("(c p) b -> p c b", p=128), ol[:]
                )

            # ---- final state out ----
            nc.sync.dma_start(hT_d.rearrange("(k p) b -> p k b", p=128), hT[:])
            nc.sync.dma_start(cT_d.rearrange("(k p) b -> p k b", p=128), cT[:])

    nc.finalize()
    return nc


def _prep_inputs(
    f_pool, ground_truth, Wx, Wh, b, softmax_w, softmax_b, t_steps=T, use_gather=True
):
    """Host-side layout prep + sharding. Returns per-core input maps."""
    f_pool = np.ascontiguousarray(np.asarray(f_pool, dtype=np.float32))
    gt = np.asarray(ground_truth)
    Wx = np.asarray(Wx, dtype=np.float32)
    Wh = np.ascontiguousarray(np.asarray(Wh, dtype=np.float32))
    b = np.asarray(b, dtype=np.float32)
    smw = np.ascontiguousarray(np.asarray(softmax_w, dtype=np.float32))
    smb = np.asarray(softmax_b, dtype=np.float32)

    wx1 = np.ascontiguousarray(Wx[:D])
    bcol = np.ascontiguousarray(b.reshape(NM, 128).T)
    smbcol = np.ascontiguousarray(smb.reshape(NCI, 128).T)

    common = {
        "wx1": wx1,
        "wh": Wh,
        "smw": smw,
        "bcol": bcol,
        "smbcol": smbcol,
    }
    if use_gather:
        import ml_dtypes

        wx2g = np.zeros((C + 16, 4 * R), dtype=ml_dtypes.bfloat16)
        wx2g[:C] = Wx[D : D + C].astype(ml_dtypes.bfloat16)
        common["wx2g"] = wx2g
    else:
        common["wx2"] = np.ascontiguousarray(Wx[D : D + C])
        iotac = (
            np.arange(128, dtype=np.float32)[:, None]
            + 128.0 * np.arange(NCI, dtype=np.float32)[None, :]
        )
        common["iotac"] = np.ascontiguousarray(iotac)
        common["ones"] = np.ones((1, 128), dtype=np.float32)

    NIDX = BC // IDXW
    in_maps = []
    for core in range(NCORES):
        b0 = core * BC
        fp = f_pool[b0 : b0 + BC, :t_steps]  # [Bc, t, D]
        featT = np.ascontiguousarray(fp.transpose(1, 2, 0))  # [t, D, Bc]
        m = dict(common)
        m["featT"] = featT
        if use_gather:
            # idx per step: t=0 -> C (zeros row); else gt[:, t-1]
            idx = np.full((t_steps, BC), C, dtype=np.int16)
            idx[1:] = gt[b0 : b0 + BC, : t_steps - 1].T.astype(np.int16)
            # wrap: pidx[p, t*NIDX + s] = idx[t, s*16 + p] for p<16,
            # replicated to all 8 GPSIMD core groups (16 partitions each)
            w = idx.reshape(t_steps, NIDX, IDXW)  # [t, s, p]
            blk = w.transpose(2, 0, 1).reshape(IDXW, t_steps * NIDX)
            m["pidx"] = np.ascontiguousarray(np.tile(blk, (128 // IDXW, 1)))
        else:
            prevT = np.full((t_steps, BC), -1.0, dtype=np.float32)
            prevT[1:] = gt[b0 : b0 + BC, : t_steps - 1].T.astype(np.float32)
            m["prevT"] = np.ascontiguousarray(prevT)
        in_maps.append(m)
    return in_maps


def _postprocess(results, t_steps=T):
    """Gather per-core outputs back to full [B,T,C], [B,R], [B,R]."""
    seq = np.empty((B, t_steps, C), dtype=np.float32)
    h = np.empty((B, R), dtype=np.float32)
    c = np.empty((B, R), dtype=np.float32)
    for core, res in enumerate(results):
        b0 = core * BC
        seq[b0 : b0 + BC] = res["outl"].transpose(2, 0, 1)
        h[b0 : b0 + BC] = res["hT_out"].T
        c[b0 : b0 + BC] = res["cT_out"].T
    return seq, h, c


def run(inputs, t_steps=T, trace=False, use_gather=True):
    from concourse import bass_utils

    key = (t_steps, use_gather)
    if key not in _CACHED:
        _CACHED[key] = _build_program(t_steps, use_gather)
    nc = _CACHED[key]
    in_maps = _prep_inputs(**inputs, t_steps=t_steps, use_gather=use_gather)
    res = bass_utils.run_bass_kernel_spmd(
        nc, in_maps, core_ids=list(range(NCORES)), trace=trace
    )
    return res


def kernel(**inputs):
    res = run(inputs)
    return _postprocess(res.results)


# revision 23
# speedup vs baseline: 1.6695x; 1.0035x over previous
"""ChaRNN (LSTM w/ teacher forcing) Trainium2 kernel.

B=4096, T=32, D=512 (per-step feature), R=512 (rnn size), C=256 (classes).
Data-parallel over batch: 8 cores x 512 rows each.

Device-side layout is fully "transposed" (batch on the free dim):
  hT, cT: [R, Bc] as SBUF [128, 4, 512]
  zT = Wx1^T @ featT + Wh^T @ hT (+ prev-char row of Wx2)
so no on-device transposes are needed anywhere.  The host pre-transposes
f_pool into featT [T, D, Bc] and post-transposes the outputs (layout prep
only; all FLOPs happen on device).  Matmuls run in float32r (full-rate
fp32 PE mode).

Teacher forcing means the prev-char one-hot contribution is a known row
gather of Wx2: done with dma_gather(transpose=True) from HBM in bf16
(index 256 = appended zeros row, used at t=0), then added to zT on DVE.
"""

import numpy as np

B, T, D, R, C = 4096, 32, 512, 512, 256
NCORES = 8
BC = B // NCORES  # 512 batch rows per core
NM = (4 * R) // 128  # 16 gate row-tiles
KF = D // 128  # 4 K-chunks for the feature matmul
KO = C // 128  # 2 K-chunks for the one-hot matmul (non-gather variant)
KH = R // 128  # 4 K-chunks for the recurrent matmul
NCI = C // 128  # 2 logits row-tiles
IDXW = 16  # dma_gather index wrap width

_CACHED = {}


def _build_program(t_steps=T, use_gather=True):
    import concourse.bacc as bacc
    import concourse.bass as bass
    import concourse.mybir as mybir
    import concourse.tile as tile

    f32 = mybir.dt.float32
    f32r = mybir.dt.float32r
    bf16 = mybir.dt.bfloat16
    i16 = mybir.dt.int16
    AF = mybir.ActivationFunctionType

    nc = bacc.Bacc()

    # ---- I/O ----
    featT_d = nc.dram_tensor("featT", [t_steps, D, BC], f32r, kind="ExternalInput")
    wx1_d = nc.dram_tensor("wx1", [D, 4 * R], f32r, kind="ExternalInput")
    wh_d = nc.dram_tensor("wh", [R, 4 * R], f32r, kind="ExternalInput")
    smw_d = nc.dram_tensor("smw", [R, C], f32r, kind="ExternalInput")
    bcol_d = nc.dram_tensor("bcol", [128, NM], f32, kind="ExternalInput")
    smbcol_d = nc.dram_tensor("smbcol", [128, NCI], f32, kind="ExternalInput")
    if use_gather:
        wx2g_d = nc.dram_tensor("wx2g", [C + 16, 4 * R], bf16, kind="ExternalInput")
        pidx_d = nc.dram_tensor(
            "pidx", [128, t_steps * (BC // IDXW)], i16, kind="ExternalInput"
        )
    else:
        wx2_d = nc.dram_tensor("wx2", [C, 4 * R], f32r, kind="ExternalInput")
        prevT_d = nc.dram_tensor("prevT", [t_steps, BC], f32r, kind="ExternalInput")
        iotac_d = nc.dram_tensor("iotac", [128, NCI], f32, kind="ExternalInput")
        ones_d = nc.dram_tensor("ones", [1, 128], f32r, kind="ExternalInput")

    outl_d = nc.dram_tensor("outl", [t_steps, C, BC], f32, kind="ExternalOutput")
    hT_d = nc.dram_tensor("hT_out", [R, BC], f32r, kind="ExternalOutput")
    cT_d = nc.dram_tensor("cT_out", [R, BC], f32, kind="ExternalOutput")

    NIDX = BC // IDXW  # 32 idx columns per step

    with tile.TileContext(nc) as tc:
        with (
            tc.tile_pool(name="weights", bufs=1) as wpool,
            tc.tile_pool(name="state", bufs=2) as spool,
            tc.tile_pool(name="feat", bufs=2) as fpool,
            tc.tile_pool(name="oh", bufs=2) as ohpool,
            tc.tile_pool(name="gates", bufs=2) as gpool,
            tc.tile_pool(name="outs", bufs=2) as opool,
            tc.tile_pool(name="zpsum", bufs=6, space="PSUM") as zpool,
            tc.tile_pool(name="lpsum", bufs=2, space="PSUM") as lpool,
        ):
            # ---- load weights (once); split per K-chunk so the first
            # matmuls only wait for their own chunk ----
            wx1 = wpool.tile([128, KF, 4 * R], f32r)
            for k in range(KF):
                eng = nc.sync if k % 2 == 0 else nc.scalar
                eng.dma_start(wx1[:, k, :], wx1_d[k * 128 : (k + 1) * 128, :])
            def load_ft(t, split=False):
                ft = fpool.tile([128, KF, BC], f32r, tag="ft")
                if split:
                    for k in range(KF):
                        eng = nc.sync if k % 2 == 0 else nc.scalar
                        eng.dma_start(
                            ft[:, k, :], featT_d[t, k * 128 : (k + 1) * 128, :]
                        )
                else:
                    nc.sync.dma_start(
                        ft[:], featT_d[t].rearrange("(k p) b -> p k b", p=128)
                    )
                return ft

            ft0 = load_ft(0)  # prefetch step-0 features before wh
            wh = wpool.tile([128, KH, 4 * R], f32r)
            for k in range(KH):
                eng = nc.sync if k % 2 == 1 else nc.scalar
                eng.dma_start(wh[:, k, :], wh_d[k * 128 : (k + 1) * 128, :])
            smw = wpool.tile([128, KH, C], f32r)
            nc.scalar.dma_start(smw[:], smw_d.rearrange("(k p) m -> p k m", p=128))
            bcol = wpool.tile([128, NM], f32)
            nc.scalar.dma_start(bcol[:], bcol_d[:])
            smbcol = wpool.tile([128, NCI], f32)
            nc.scalar.dma_start(smbcol[:], smbcol_d[:])
            if use_gather:
                pidx = wpool.tile([128, t_steps * NIDX], i16)
                nc.scalar.dma_start(pidx[:], pidx_d[:])
            else:
                wx2 = wpool.tile([128, KO, 4 * R], f32r)
                for k in range(KO):
                    nc.sync.dma_start(wx2[:, k, :], wx2_d[k * 128 : (k + 1) * 128, :])
                iotac = wpool.tile([128, NCI], f32)
                nc.sync.dma_start(iotac[:], iotac_d[:])
                ones = wpool.tile([1, 128], f32r)
                nc.sync.dma_start(ones[:], ones_d[:])

            # ---- initial state: h0 = c0 = 0, handled by skipping the
            # recurrent matmuls and the zero-row gather at t=0 ----
            hT = None
            cT = spool.tile([128, KH, BC], f32, tag="cT")
            nc.vector.memset(cT[:], 0.0)

            for t in range(t_steps):
                if use_gather and t > 0:
                    # prev-char rows of Wx2, transposed by the gather DMA:
                    # z2t[p, m, j] = Wx2g[idx_j, m*128+p]  (bf16)
                    z2t = ohpool.tile([128, NM, BC], bf16, tag="z2t")
                    nc.gpsimd.dma_gather(
                        out_ap=z2t[:],
                        in_ap=wx2g_d[:],
                        idxs_ap=pidx[:, t * NIDX : (t + 1) * NIDX],
                        num_idxs=BC,
                        num_idxs_reg=BC,
                        elem_size=4 * R,
                        transpose=True,
                    )
                elif use_gather:
                    z2t = None
                else:
                    prow = ohpool.tile([1, BC], f32r, tag="prow")
                    nc.sync.dma_start(prow[:], prevT_d[t : t + 1, :])
                    pb = lpool.tile([128, BC], f32, tag="pb")
                    nc.tensor.matmul(pb[:], ones[:], prow[:], start=True, stop=True)
                    oh = ohpool.tile([128, KO, BC], f32r, tag="oh")
                    for ci in range(KO):
                        nc.vector.tensor_scalar(
                            oh[:, ci, :],
                            pb[:],
                            iotac[:, ci : ci + 1],
                            None,
                            mybir.AluOpType.is_equal,
                        )

                # per-step image features, pre-transposed on host
                ft = ft0 if t == 0 else load_ft(t)

                # ---- zT accumulation: 16 row-tiles of [128, Bc] ----
                zts = []
                for m in range(NM):
                    zt = zpool.tile([128, BC], f32, tag="zt")
                    zts.append(zt)
                    for k in range(KF):
                        nc.tensor.matmul(
                            zt[:],
                            wx1[:, k, m * 128 : (m + 1) * 128],
                            ft[:, k, :],
                            start=(k == 0),
                            stop=(t == 0 and use_gather and k == KF - 1),
                        )
                    if not use_gather:
                        for k in range(KO):
                            nc.tensor.matmul(
                                zt[:],
                                wx2[:, k, m * 128 : (m + 1) * 128],
                                oh[:, k, :],
                                start=False,
                                stop=False,
                            )
                # recurrent part (needs previous h); h(-1) = 0 -> skip at t=0
                if t > 0:
                    for m in range(NM):
                        for k in range(KH):
                            nc.tensor.matmul(
                                zts[m][:],
                                wh[:, k, m * 128 : (m + 1) * 128],
                                hT[:, k, :],
                                start=False,
                                stop=(k == KH - 1),
                            )

                # ---- gates (Keras order i, f, g, o along rows) ----
                hT_new = spool.tile([128, KH, BC], f32r, tag="hT")
                cT_new = spool.tile([128, KH, BC], f32, tag="cT")
                for j in range(KH):  # 4 chunks of 128 R-rows
                    mi, mf, mg, mo = j, 4 + j, 8 + j, 12 + j
                    it = gpool.tile([128, BC], f32, tag="it")
                    ftg = gpool.tile([128, BC], f32, tag="ftg")
                    gt = gpool.tile([128, BC], f32, tag="gt")
                    ot = gpool.tile([128, BC], f32, tag="ot")
                    if use_gather and t > 0:
                        # z = zT(psum) + gathered prev-char row, then gate fn
                        zi = gpool.tile([128, BC], f32, tag="zi")
                        zf = gpool.tile([128, BC], f32, tag="zf")
                        zg = gpool.tile([128, BC], f32, tag="zg")
                        zo = gpool.tile([128, BC], f32, tag="zo")
                        nc.vector.tensor_add(zi[:], zts[mi][:], z2t[:, mi, :])
                        nc.vector.tensor_add(zf[:], zts[mf][:], z2t[:, mf, :])
                        nc.vector.tensor_add(zg[:], zts[mg][:], z2t[:, mg, :])
                        nc.vector.tensor_add(zo[:], zts[mo][:], z2t[:, mo, :])
                        si, sf, sg, so = zi, zf, zg, zo
                    else:
                        si, sf, sg, so = zts[mi], zts[mf], zts[mg], zts[mo]
                    nc.scalar.activation(
                        it[:], si[:], AF.Sigmoid, bias=bcol[:, mi : mi + 1]
                    )
                    nc.scalar.activation(
                        ftg[:], sf[:], AF.Sigmoid, bias=bcol[:, mf : mf + 1]
                    )
                    nc.scalar.activation(
                        gt[:], sg[:], AF.Tanh, bias=bcol[:, mg : mg + 1]
                    )
                    nc.scalar.activation(
                        ot[:], so[:], AF.Sigmoid, bias=bcol[:, mo : mo + 1]
                    )
                    if t == 0:
                        nc.vector.tensor_mul(cT_new[:, j, :], it[:], gt[:])
                    else:
                        ig = gpool.tile([128, BC], f32, tag="ig")
                        nc.vector.tensor_mul(ig[:], it[:], gt[:])
                        nc.vector.tensor_mul(cT_new[:, j, :], cT[:, j, :], ftg[:])
                        nc.vector.tensor_add(cT_new[:, j, :], cT_new[:, j, :], ig[:])
                    tct = gpool.tile([128, BC], f32, tag="tct")
                    nc.scalar.activation(tct[:], cT_new[:, j, :], AF.Tanh)
                    nc.vector.tensor_mul(hT_new[:, j, :], ot[:], tct[:])
                hT, cT = hT_new, cT_new

                # ---- logits: [C, Bc] in 2 row-tiles ----
                ol = opool.tile([128, NCI, BC], f32, tag="ol")
                for ci in range(NCI):
                    lg = lpool.tile([128, BC], f32, tag="lg")
                    for k in range(KH):
                        nc.tensor.matmul(
                            lg[:],
                            smw[:, k, ci * 128 : (ci + 1) * 128],
                            hT[:, k, :],
                            start=(k == 0),
                            stop=(k == KH - 1),
                        )
                    nc.scalar.activation(
                        ol[:, ci, :], lg[:], AF.Identity, bias=smbcol[:, ci : ci + 1]
                    )
                nc.sync.dma_start(
                    outl_d[t].rearrange("(c p) b -> p c b", p=128), ol[:]
                )

            # ---- final state out (per chunk, so stores overlap the
            # remaining gate computation of the last step) ----
            for j in range(KH):
                nc.sync.dma_start(hT_d[j * 128 : (j + 1) * 128, :], hT[:, j, :])
                nc.scalar.dma_start(cT_d[j * 128 : (j + 1) * 128, :], cT[:, j, :])

    nc.finalize()
    return nc


def _prep_inputs(
    f_pool, ground_truth, Wx, Wh, b, softmax_w, softmax_b, t_steps=T, use_gather=True
):
    """Host-side layout prep + sharding. Returns per-core input maps."""
    f_pool = np.ascontiguousarray(np.asarray(f_pool, dtype=np.float32))
    gt = np.asarray(ground_truth)
    Wx = np.asarray(Wx, dtype=np.float32)
    Wh = np.ascontiguousarray(np.asarray(Wh, dtype=np.float32))
    b = np.asarray(b, dtype=np.float32)
    smw = np.ascontiguousarray(np.asarray(softmax_w, dtype=np.float32))
    smb = np.asarray(softmax_b, dtype=np.float32)

    wx1 = np.ascontiguousarray(Wx[:D])
    bcol = np.ascontiguousarray(b.reshape(NM, 128).T)
    smbcol = np.ascontiguousarray(smb.reshape(NCI, 128).T)

    common = {
        "wx1": wx1,
        "wh": Wh,
        "smw": smw,
        "bcol": bcol,
        "smbcol": smbcol,
    }
    if use_gather:
        import ml_dtypes

        wx2g = np.zeros((C + 16, 4 * R), dtype=ml_dtypes.bfloat16)
        wx2g[:C] = Wx[D : D + C].astype(ml_dtypes.bfloat16)
        common["wx2g"] = wx2g
    else:
        common["wx2"] = np.ascontiguousarray(Wx[D : D + C])
        iotac = (
            np.arange(128, dtype=np.float32)[:, None]
            + 128.0 * np.arange(NCI, dtype=np.float32)[None, :]
        )
        common["iotac"] = np.ascontiguousarray(iotac)
        common["ones"] = np.ones((1, 128), dtype=np.float32)

    NIDX = BC // IDXW
    in_maps = []
    for core in range(NCORES):
        b0 = core * BC
        fp = f_pool[b0 : b0 + BC, :t_steps]  # [Bc, t, D]
        featT = np.ascontiguousarray(fp.transpose(1, 2, 0))  # [t, D, Bc]
        m = dict(common)
        m["featT"] = featT
        if use_gather:
            # idx per step: t=0 -> C (zeros row); else gt[:, t-1]
            idx = np.full((t_steps, BC), C, dtype=np.int16)
            idx[1:] = gt[b0 : b0 + BC, : t_steps - 1].T.astype(np.int16)
            # wrap: pidx[p, t*NIDX + s] = idx[t, s*16 + p] for p<16,
            # replicated to all 8 GPSIMD core groups (16 partitions each)
            w = idx.reshape(t_steps, NIDX, IDXW)  # [t, s, p]
            blk = w.transpose(2, 0, 1).reshape(IDXW, t_steps * NIDX)
            m["pidx"] = np.ascontiguousarray(np.tile(blk, (128 // IDXW, 1)))
        else:
            prevT = np.full((t_steps, BC), -1.0, dtype=np.float32)
            prevT[1:] = gt[b0 : b0 + BC, : t_steps - 1].T.astype(np.float32)
            m["prevT"] = np.ascontiguousarray(prevT)
        in_maps.append(m)
    return in_maps


def _postprocess(results, t_steps=T):
    """Gather per-core outputs back to full [B,T,C], [B,R], [B,R]."""
    seq = np.empty((B, t_steps, C), dtype=np.float32)
    h = np.empty((B, R), dtype=np.float32)
    c = np.empty((B, R), dtype=np.float32)
    for core, res in enumerate(results):
        b0 = core * BC
        seq[b0 : b0 + BC] = res["outl"].transpose(2, 0, 1)
        h[b0 : b0 + BC] = res["hT_out"].T
        c[b0 : b0 + BC] = res["cT_out"].T
    return seq, h, c


def run(inputs, t_steps=T, trace=False, use_gather=True):
    from concourse import bass_utils

    key = (t_steps, use_gather)
    if key not in _CACHED:
        _CACHED[key] = _build_program(t_steps, use_gather)
    nc = _CACHED[key]
    in_maps = _prep_inputs(**inputs, t_steps=t_steps, use_gather=use_gather)
    res = bass_utils.run_bass_kernel_spmd(
        nc, in_maps, core_ids=list(range(NCORES)), trace=trace
    )
    return res


def kernel(**inputs):
    res = run(inputs)
    return _postprocess(res.results)
